# revision 43
# baseline (speedup 1.0000x reference)
"""Trainium2 Bass kernel for nn_CombinatorialClassifier.

Computation (reference):
    logits = einsum('bf,pqf->bpq', x, W) + b        # [B,P,Q]
    logp   = log_softmax(logits, axis=2)            # [B,P,Q]
    out    = take_along_axis(logp, part_idx, 2)     # [B,P,C]

Shapes: B=256, P=64, Q=128, C=1000, F=2048.  Expert-parallel over P
across 8 cores (PL=8 partitionings per core), no collectives.

Structure (measured ~52-56us vs the 76us q-orientation baseline):
  - main matmul in "b-orientation": stationary = xT k-slab [128f, 128b],
    moving = W k-slab [128f, (p,q)-chunk] -> psum_lin[b, (p,q)], both
    fp8e4 with DoubleRow (K=256/matmul, 2 MACs/cell/cycle): 32 N=512
    matmuls total.  W (x64 pre-scale, folded back out in the softmax)
    streams on the sync HWDGE family; ident/x/idx ride the scalar
    family.  DMA waits are schedule-order coarse (a consumer waits for
    every family-DMA scheduled before it), so the idx issues are pinned
    mid-main with scheduler hints.
  - PE HAM warm-up: the clock gate defaults to 1.2 GHz and watches PE
    ARRAY activity (K=1 matmuls read as idle!), so full-K junk matmuls
    off the early ident DMA warm it before the main phase; bt-outer
    main ordering keeps the PE dense enough afterwards to stay warm.
  - softmax chain on ACT (Exp -> DVE reduce -> Ln -> negate -> linY
    cast), pinned so bt0's chain precedes exp1 on the ACT queue; lse is
    subtracted at DRAIN time: DVE drains fuse (po/64 - lse) in one
    scalar_tensor_tensor, ACT drains use Identity with per-partition
    bias AP (ACT drains measure ~0.69us vs DVE 1.2us -> ACT takes 10).
  - gather: logits transposed back to [q, b] by 8 PE transposes per bt
    into one bf16 PSUM tile, then logpT.T @ one-hot; one-hot = 8 DVE
    is_equal ops in 2x mode against a host-built iota block, DMA-
    replicated across partitions (fp16).
  - single 4-slot PSUM pool (8 banks); drain engines colored by p%4
    class so the stride-4 slot rotation gives each gather a WAR partner
    drained by its own input engine (single-semaphore waits).
  - output bf16 (tolerance 2e-2 >> bf16 eps, halves out-DMA), 16 per-p
    out-DMAs on the sync family which is idle in the tail.
  - _install_wait_split legalizes remaining multi-wait instructions
    (this walrus build: max one sync-wait/instruction) by prepending
    same-engine wait-only Drains.
"""

import numpy as np

B, P, Q, C, F = 256, 64, 128, 1000, 2048
NCORES = 8
PL = P // NCORES          # partitionings per core
KT = F // 128             # contraction k-tiles
KC = 8                    # k-tiles per W DMA chunk
SCALE = 64.0              # W pre-scale keeps fp8e4 out of subnormals
N_WARM = 22               # junk matmuls at t=0 (PE HAM warm-up).  They
                          # MUST be full-K: the HAM clock gate watches
                          # PE array activity, and a K=1 matmul lights
                          # up 1 of 128 rows (reads as idle)


def _build_nc():
    import concourse.bass as bass
    import concourse.tile as tile
    from concourse import mybir
    from contextlib import ExitStack

    F32 = mybir.dt.float32
    BF16 = mybir.dt.bfloat16
    FP16 = mybir.dt.float16
    FP8 = mybir.dt.float8e4
    AF = mybir.ActivationFunctionType
    ALU = mybir.AluOpType

    nc = bass.Bass()
    bias_d = nc.declare_dram_parameter("biasr", [1, PL * Q + 128], BF16,
                                       isOutput=False)
    xT_d = nc.declare_dram_parameter("xT", [128, KT // 2, 2, 256], FP8,
                                     isOutput=False)
    id_d = nc.declare_dram_parameter("ident", [128, 128], BF16,
                                     isOutput=False)
    wm_d = nc.declare_dram_parameter(
        "wm", [KT // KC, 128, KC // 2, 2, PL * Q], FP8, isOutput=False)
    # cols [0,1000) = iota (row q has value q), [1000+p*1000, ...) = idx[p]
    idx_d = nc.declare_dram_parameter("idxq", [128, (PL + 1) * C], FP16,
                                      isOutput=False)
    out_d = nc.declare_dram_parameter("out", [B, PL, C], BF16, isOutput=True)

    with ExitStack() as ctx:
        tc = ctx.enter_context(tile.TileContext(nc))
        sb = ctx.enter_context(tc.tile_pool(name="sb", bufs=1))
        # one pool, 4 slots x 2 banks = all 8 PSUM banks; warmup target,
        # lin_bt0/1, transpose targets and gather outputs all rotate here
        ps = ctx.enter_context(
            tc.tile_pool(name="ps", bufs=4, space=bass.MemorySpace.PSUM))

        def fresh(shape, dtype, tag):
            return sb.tile(shape, dtype, tag=tag, name=tag)

        # ---- input DMAs ----------------------------------------------
        # sync family: bias -> W chunks ONLY (consumers' DMA waits are
        # schedule-order coarse: anything on the family issued before a
        # consumer gates it, so idx must not share the W family).
        # scalar family: ident (warm-up operand, first), x, then idx --
        # with scheduler hints so the idx issues sit AFTER the first
        # main matmul and never gate the x wait.
        biasr = fresh([1, PL * Q + 128], BF16, "biasr")
        nc.sync.dma_start(out=biasr[:], in_=bias_d[:])
        ident = fresh([128, 128], BF16, "ident")
        nc.scalar.dma_start(out=ident[:], in_=id_d[:])
        xT = fresh([128, KT // 2, 2, 256], FP8, "xT")
        xT_dma = nc.scalar.dma_start(out=xT[:], in_=xT_d[:])
        wkc = []
        w_dmas = []
        for j in range(KT // KC):
            t = fresh([128, KC // 2, 2, PL * Q], FP8, f"wk{j}")
            w_dmas.append(nc.sync.dma_start(out=t[:], in_=wm_d[j]))
            wkc.append(t)
        idx_sb = fresh([128, (PL + 1) * C], FP16, "idxq")
        idx_dma = [
            nc.scalar.dma_start(out=idx_sb[:, 0:5 * C],
                                in_=idx_d[:, 0:5 * C]),
            nc.scalar.dma_start(out=idx_sb[:, 5 * C:],
                                in_=idx_d[:, 5 * C:]),
        ]

        # ---- PE warm-up: full-K junk matmuls off a DVE memset --------
        # (must be full-K: the HAM clock gate watches PE array activity
        # and a K=1 matmul reads as idle.  A memset-fed tile lets the
        # warm-up start ~7us in, before any DMA has landed)
        ones_ap = biasr[:, PL * Q:PL * Q + 128]
        wu_ps = ps.tile([128, 1024], F32, tag="ps", name="wu_ps")
        wu = fresh([128, 512], BF16, "wu")
        nc.vector.memset(wu[:], 1.0)
        warm_mms = []
        for _ in range(N_WARM):
            warm_mms.append(
                nc.tensor.matmul(wu_ps[:, 0:512], wu[:, 0:128], wu[:, :],
                                 start=True, stop=True))

        # ---- one-hot per p: 2x-mode DVE is_equal against iota --------
        oh = []
        oh_ops = []
        for p in range(PL):
            t = fresh([128, C], BF16, f"oh{p}")
            oh_ops.append(nc.vector.tensor_tensor(
                out=t[:], in0=idx_sb[:, (1 + p) * C:(2 + p) * C],
                in1=idx_sb[:, 0:C], op=ALU.is_equal))
            oh.append(t)

        # ---- main matmuls: psum_lin[b, (p,q)] ------------------------
        # DoubleRow fp8: k-tiles are paired (K=256 per matmul, 2 MACs/
        # cell/cycle) halving the matmul+LDWEIGHTS count.  xT is laid
        # out [128f, T, ko, 256b], W chunks [128f, tt, ko, 1024pq].
        # bt-OUTER: bt0's accumulation completes ~5us before bt1's, so
        # its softmax chain and tail overlap bt1's main matmuls; the
        # PE order is pinned with a scheduler hint (Tile otherwise
        # interleaves the two bt's matmul groups).
        # The whole softmax chain lives on ACT: Exp with accum_out
        # replaces the DVE reduce, and lse is subtracted at DRAIN time
        # (DVE drains: fused scalar_tensor_tensor; ACT drains: Identity
        # with per-partition bias) so DVE only builds one-hots and
        # drains.
        DR = mybir.MatmulPerfMode.DoubleRow
        lin = [ps.tile([128, PL, 128], F32, tag="ps", name=f"lin{bt}")
               for bt in (0, 1)]
        linY = [None, None]
        lses = [None, None]
        nlses = [None, None]
        last_mm = [None]
        first_mm = [None]

        main_mms = []

        def emit_main(bt):
            for ch in (0, 1):
                mm = nc.tensor.matmul(
                    lin[bt][:, ch * 4:(ch + 1) * 4, :],
                    ones_ap, biasr[:, ch * 512:(ch + 1) * 512],
                    start=True, stop=False)
                if bt == 0 and first_mm[0] is None:
                    first_mm[0] = mm
                if bt == 1 and last_mm[0] is not None:
                    tile.add_dep_helper(mm.ins, last_mm[0].ins, sync=False,
                                        reason="bt0 main before bt1 main")
            for t in range(KT // 2):
                j, tt = t // (KC // 2), t % (KC // 2)
                for ch in (0, 1):
                    mm = nc.tensor.matmul(
                        lin[bt][:, ch * 4:(ch + 1) * 4, :],
                        xT[:, t, :, bt * 128:(bt + 1) * 128],
                        wkc[j][:, tt, :, ch * 512:(ch + 1) * 512],
                        start=False, stop=(t == KT // 2 - 1),
                        perf_mode=DR)
                    main_mms.append(mm)
            last_mm[0] = mm

        reduce_ops = []
        chain_ops = []

        def emit_chain(bt):
            exps = fresh([128, PL, 128], BF16, f"exps{bt}")
            e = nc.scalar.activation(out=exps[:], in_=lin[bt][:],
                                     func=AF.Exp, scale=1.0 / SCALE)
            sums = fresh([128, PL], F32, f"sums{bt}")
            reduce_ops.append(
                nc.vector.tensor_reduce(out=sums[:], in_=exps[:],
                                        axis=mybir.AxisListType.X,
                                        op=ALU.add))
            ly = fresh([128, PL, 128], BF16, f"linY{bt}")
            lyop = nc.scalar.activation(out=ly[:], in_=lin[bt][:],
                                        func=AF.Copy)
            lse = fresh([128, PL], F32, f"lse{bt}")
            nc.scalar.activation(out=lse[:], in_=sums[:], func=AF.Ln)
            nlse = fresh([128, PL], F32, f"nlse{bt}")
            nc.scalar.mul(nlse[:], lse[:], -1.0)
            linY[bt] = ly
            lses[bt] = lse
            nlses[bt] = nlse
            chain_ops.append((e, lyop))

        emit_main(0)
        emit_chain(0)
        emit_main(1)
        emit_chain(1)

        # bt0's whole ACT chain (Ln/neg/linY) must precede exp1 on the
        # ACT queue, else Ln0 gets stuck behind exp1's data wait
        tile.add_dep_helper(chain_ops[1][0].ins, chain_ops[0][1].ins,
                            sync=False, reason="act order")
        # idx issues scheduled mid-main so the main matmuls' DMA wait
        # targets exclude idx, but idx still lands before the gathers
        tile.add_dep_helper(w_dmas[1].ins, main_mms[0].ins, sync=False,
                            reason="W1 issue after main start")
        for dma in idx_dma:
            tile.add_dep_helper(dma.ins, main_mms[12].ins, sync=False,
                                reason="idx issue mid-main")
        # pin the DVE queue so the softmax reduces are not stuck behind
        # all eight one-hot builds
        tile.add_dep_helper(reduce_ops[0].ins, oh_ops[3].ins, sync=False,
                            reason="dve order")
        tile.add_dep_helper(oh_ops[4].ins, reduce_ops[0].ins, sync=False,
                            reason="dve order")
        tile.add_dep_helper(reduce_ops[1].ins, oh_ops[4].ins, sync=False,
                            reason="dve order")
        tile.add_dep_helper(oh_ops[5].ins, reduce_ops[1].ins, sync=False,
                            reason="dve order")

        # ---- per-bt: transpose -> gather -> drain -> out DMA ---------
        # drain/linT engine coloring by p%4 class (stride-4 PSUM slot
        # rotation makes a gather's WAR partner share its class, so its
        # input dep and WAR dep land on one semaphore).  ACT drains
        # measure ~0.69us vs DVE's ~1.2us, so ACT takes 10 of 16.
        def drain_dve(bt, p):
            return (p % 4) < 2 if bt == 0 else (p % 4) == 0

        for bt in (0, 1):
            tr = ps.tile([128, 2, 4, 128], BF16, tag="ps", name=f"tr{bt}")
            for p in range(PL):
                nc.tensor.transpose(tr[:, p // 4, p % 4, :],
                                    linY[bt][:, p, :], ident[:])
            linT = fresh([128, 2, 4, 128], BF16, f"linT{bt}")
            if bt == 0:
                lt_dve = nc.vector.tensor_copy(out=linT[:, :, 0:2, :],
                                               in_=tr[:, :, 0:2, :])
                lt_act = nc.scalar.activation(out=linT[:, :, 2:4, :],
                                              in_=tr[:, :, 2:4, :],
                                              func=AF.Copy)
                # unblock bt0's gathers: exp1 (chain1) yields the ACT
                # queue to linT0's ACT half; oh5..7 yield DVE likewise
                tile.add_dep_helper(chain_ops[1][0].ins, lt_act.ins,
                                    sync=False, reason="linT0 before exp1")
                tile.add_dep_helper(oh_ops[5].ins, lt_dve.ins,
                                    sync=False, reason="linT0 before oh5")
            else:
                nc.vector.tensor_copy(out=linT[:, :, 0, :],
                                      in_=tr[:, :, 0, :])
                nc.scalar.activation(out=linT[:, :, 1, :],
                                     in_=tr[:, :, 1, :], func=AF.Copy)
                nc.scalar.activation(out=linT[:, :, 2:4, :],
                                     in_=tr[:, :, 2:4, :], func=AF.Copy)

            for p in range(PL):
                po = ps.tile([128, 1024], F32, tag="ps", name=f"po{bt}_{p}")
                lt = linT[:, p // 4, p % 4, :]
                nc.tensor.matmul(po[:, 0:512], lt, oh[p][:, 0:512],
                                 start=True, stop=True)
                nc.tensor.matmul(po[:, 512:1000], lt, oh[p][:, 512:1000],
                                 start=True, stop=True)
                g = fresh([128, C], BF16, f"og{bt}_{p}")
                # drain = gathered 64*logit / 64 - lse, fused per engine
                if drain_dve(bt, p):
                    nc.vector.scalar_tensor_tensor(
                        out=g[:], in0=po[:, 0:1000],
                        scalar=1.0 / SCALE,
                        in1=lses[bt][:, p:p + 1].broadcast_to((128, C)),
                        op0=ALU.mult, op1=ALU.subtract)
                else:
                    nc.scalar.activation(
                        out=g[:], in_=po[:, 0:1000],
                        func=AF.Identity, scale=1.0 / SCALE,
                        bias=nlses[bt][:, p:p + 1])
                bsl = slice(bt * 128, (bt + 1) * 128)
                nc.sync.dma_start(out=out_d[bsl, p:p + 1, :], in_=g[:])

    _install_wait_split(nc)
    return nc


def _install_wait_split(nc):
    """This walrus build encodes at most ONE sync-wait per instruction.
    Legalize at serialization time: any instruction carrying N>1 waits
    gets N-1 wait-only Drain instructions (same engine, so the queue
    stalls identically) inserted in front of it; the instruction keeps
    the last wait.  Semantically identical (serial sem waits)."""
    import json

    orig = nc.to_json_bytes

    def patched():
        m = json.loads(orig())
        for fn in m["functions"]:
            for bb in fn["blocks"]:
                out = []
                for inst in bb["instructions"]:
                    si = inst.get("sync_info")
                    waits = (si or {}).get("on_wait") or []
                    if len(waits) > 1:
                        head, keep = waits[:-1], waits[-1:]
                        for j, w in enumerate(head):
                            out.append({
                                "engine": inst["engine"],
                                "ins": [],
                                "outs": [],
                                "name": f"{inst['name']}-ws{j}",
                                "opcode": "Drain",
                                "sync_info": {
                                    "on_wait": [w],
                                    "on_update": [],
                                },
                            })
                        si["on_wait"] = keep
                    out.append(inst)
                bb["instructions"] = out
        return json.dumps(m).encode()

    nc.to_json_bytes = patched


def _host_inputs(x, W, b, part_idx):
    import ml_dtypes

    f8 = ml_dtypes.float8_e4m3
    bf = ml_dtypes.bfloat16

    # xT[f_sub, t, ko, b] = x[b, (2t+ko)*128 + f_sub]  (DoubleRow pairs)
    xT = np.ascontiguousarray(
        x.reshape(B, KT, 128).transpose(2, 1, 0)       # [128, KT, B]
        .reshape(128, KT // 2, 2, B)).astype(f8)
    ident = np.eye(128, dtype=np.float32).astype(bf)
    iota = np.arange(128, dtype=np.float16)

    in_maps = []
    for i in range(NCORES):
        sl = slice(i * PL, (i + 1) * PL)
        # wm[j, f_sub, tt, ko, p*128+q] = SCALE * W[p, q, k*128+f_sub]
        # with k = j*KC + 2*tt + ko  (DoubleRow pairs)
        wm = np.ascontiguousarray(
            (W[sl] * SCALE).transpose(2, 0, 1)          # [F, PL, Q]
            .reshape(KT // KC, KC, 128, PL * Q)
            .transpose(0, 2, 1, 3)                      # [J, 128, KC, PL*Q]
            .reshape(KT // KC, 128, KC // 2, 2, PL * Q)).astype(f8)
        biasr = np.empty((1, PL * Q + 128), dtype=bf)
        biasr[0, :PL * Q] = (b[sl] * SCALE).reshape(-1).astype(bf)
        biasr[0, PL * Q:] = 1.0
        idxq = np.empty((128, (PL + 1) * C), dtype=np.float16)
        idxq[:, 0:C] = iota[:, None]
        idxq[:, C:] = np.broadcast_to(
            part_idx[sl].astype(np.float16).reshape(1, PL * C),
            (128, PL * C))
        in_maps.append({"xT": xT, "biasr": biasr, "ident": ident,
                        "wm": wm, "idxq": idxq})
    return in_maps


def kernel(x, W, b, part_idx, _trace=False):
    from concourse.bass_utils import run_bass_kernel_spmd

    x = np.asarray(x, dtype=np.float32)
    W = np.asarray(W, dtype=np.float32)
    b = np.asarray(b, dtype=np.float32)
    part_idx = np.asarray(part_idx)

    nc = _build_nc()
    in_maps = _host_inputs(x, W, b, part_idx)
    res = run_bass_kernel_spmd(nc, in_maps, list(range(NCORES)),
                               trace=_trace)
    out = np.concatenate(
        [np.asarray(r["out"], dtype=np.float32) for r in res.results], axis=1)
    if _trace:
        return out, res
    return out


# revision 44
# speedup vs baseline: 1.0519x; 1.0519x over previous
"""Trainium2 Bass kernel for nn_CombinatorialClassifier.

Computation (reference):
    logits = einsum('bf,pqf->bpq', x, W) + b        # [B,P,Q]
    logp   = log_softmax(logits, axis=2)            # [B,P,Q]
    out    = take_along_axis(logp, part_idx, 2)     # [B,P,C]

Shapes: B=256, P=64, Q=128, C=1000, F=2048.  Expert-parallel over P
across 8 cores (PL=8 partitionings per core), no collectives.

Structure (measured ~52-56us vs the 76us q-orientation baseline):
  - main matmul in "b-orientation": stationary = xT k-slab [128f, 128b],
    moving = W k-slab [128f, (p,q)-chunk] -> psum_lin[b, (p,q)], both
    fp8e4 with DoubleRow (K=256/matmul, 2 MACs/cell/cycle): 32 N=512
    matmuls total.  W (x64 pre-scale, folded back out in the softmax)
    streams on the sync HWDGE family; ident/x/idx ride the scalar
    family.  DMA waits are schedule-order coarse (a consumer waits for
    every family-DMA scheduled before it), so the idx issues are pinned
    mid-main with scheduler hints.
  - PE HAM warm-up: the clock gate defaults to 1.2 GHz and watches PE
    ARRAY activity (K=1 matmuls read as idle!), so full-K junk matmuls
    off the early ident DMA warm it before the main phase; bt-outer
    main ordering keeps the PE dense enough afterwards to stay warm.
  - softmax chain on ACT (Exp -> DVE reduce -> Ln -> negate -> linY
    cast), pinned so bt0's chain precedes exp1 on the ACT queue; lse is
    subtracted at DRAIN time: DVE drains fuse (po/64 - lse) in one
    scalar_tensor_tensor, ACT drains use Identity with per-partition
    bias AP (ACT drains measure ~0.69us vs DVE 1.2us -> ACT takes 10).
  - gather: logits transposed back to [q, b] by 8 PE transposes per bt
    into one bf16 PSUM tile, then logpT.T @ one-hot; one-hot = 8 DVE
    is_equal ops in 2x mode against a host-built iota block, DMA-
    replicated across partitions (fp16).
  - single 4-slot PSUM pool (8 banks); drain engines colored by p%4
    class so the stride-4 slot rotation gives each gather a WAR partner
    drained by its own input engine (single-semaphore waits).
  - output bf16 (tolerance 2e-2 >> bf16 eps, halves out-DMA), 16 per-p
    out-DMAs on the sync family which is idle in the tail.
  - _install_wait_split legalizes remaining multi-wait instructions
    (this walrus build: max one sync-wait/instruction) by prepending
    same-engine wait-only Drains.
"""

import numpy as np

B, P, Q, C, F = 256, 64, 128, 1000, 2048
NCORES = 8
PL = P // NCORES          # partitionings per core
KT = F // 128             # contraction k-tiles
KC = 8                    # k-tiles per W DMA chunk
SCALE = 64.0              # W pre-scale keeps fp8e4 out of subnormals
N_WARM = 12               # junk matmuls at t=0 (PE HAM warm-up).  They
                          # MUST be full-K: the HAM clock gate watches
                          # PE array activity, and a K=1 matmul lights
                          # up 1 of 128 rows (reads as idle)


def _build_nc():
    import concourse.bass as bass
    import concourse.tile as tile
    from concourse import mybir
    from contextlib import ExitStack

    F32 = mybir.dt.float32
    BF16 = mybir.dt.bfloat16
    FP16 = mybir.dt.float16
    FP8 = mybir.dt.float8e4
    AF = mybir.ActivationFunctionType
    ALU = mybir.AluOpType

    nc = bass.Bass()
    bias_d = nc.declare_dram_parameter("biasr", [1, PL * Q + 128], BF16,
                                       isOutput=False)
    xT_d = nc.declare_dram_parameter("xT", [128, KT // 2, 2, 256], FP8,
                                     isOutput=False)
    id_d = nc.declare_dram_parameter("ident", [128, 128], BF16,
                                     isOutput=False)
    wm_d = nc.declare_dram_parameter(
        "wm", [KT // KC, 128, KC // 2, 2, PL * Q], FP8, isOutput=False)
    # cols [0,1000) = iota (row q has value q), [1000+p*1000, ...) = idx[p]
    idx_d = nc.declare_dram_parameter("idxq", [128, (PL + 1) * C], FP16,
                                      isOutput=False)
    out_d = nc.declare_dram_parameter("out", [B, PL, C], BF16, isOutput=True)

    with ExitStack() as ctx:
        tc = ctx.enter_context(tile.TileContext(nc))
        sb = ctx.enter_context(tc.tile_pool(name="sb", bufs=1))
        # one pool, 4 slots x 2 banks = all 8 PSUM banks; warmup target,
        # lin_bt0/1, transpose targets and gather outputs all rotate here
        ps = ctx.enter_context(
            tc.tile_pool(name="ps", bufs=4, space=bass.MemorySpace.PSUM))

        def fresh(shape, dtype, tag):
            return sb.tile(shape, dtype, tag=tag, name=tag)

        # ---- input DMAs ----------------------------------------------
        # sync family: bias -> W chunks ONLY (consumers' DMA waits are
        # schedule-order coarse: anything on the family issued before a
        # consumer gates it, so idx must not share the W family).
        # scalar family: ident (warm-up operand, first), x, then idx --
        # with scheduler hints so the idx issues sit AFTER the first
        # main matmul and never gate the x wait.
        biasr = fresh([1, PL * Q + 128], BF16, "biasr")
        nc.sync.dma_start(out=biasr[:], in_=bias_d[:])
        ident = fresh([128, 128], BF16, "ident")
        nc.scalar.dma_start(out=ident[:], in_=id_d[:])
        xT = fresh([128, KT // 2, 2, 256], FP8, "xT")
        xT_dma = nc.scalar.dma_start(out=xT[:], in_=xT_d[:])
        wkc = []
        w_dmas = []
        for j in range(KT // KC):
            t = fresh([128, KC // 2, 2, PL * Q], FP8, f"wk{j}")
            w_dmas.append(nc.sync.dma_start(out=t[:], in_=wm_d[j]))
            wkc.append(t)
        idx_sb = fresh([128, (PL + 1) * C], FP16, "idxq")
        idx_dma = [
            nc.scalar.dma_start(out=idx_sb[:, 0:5 * C],
                                in_=idx_d[:, 0:5 * C]),
            nc.scalar.dma_start(out=idx_sb[:, 5 * C:],
                                in_=idx_d[:, 5 * C:]),
        ]

        # ---- PE warm-up: full-K junk matmuls off a DVE memset --------
        # (must be full-K: the HAM clock gate watches PE array activity
        # and a K=1 matmul reads as idle.  A memset-fed tile lets the
        # warm-up start ~7us in, before any DMA has landed)
        ones_ap = biasr[:, PL * Q:PL * Q + 128]
        wu_ps = ps.tile([128, 1024], F32, tag="ps", name="wu_ps")
        wu = fresh([128, 512], BF16, "wu")
        nc.vector.memset(wu[:], 1.0)
        warm_mms = []
        for _ in range(N_WARM):
            warm_mms.append(
                nc.tensor.matmul(wu_ps[:, 0:512], wu[:, 0:128], wu[:, :],
                                 start=True, stop=True))

        # ---- one-hot per p: 2x-mode DVE is_equal against iota --------
        oh = []
        oh_ops = []
        for p in range(PL):
            t = fresh([128, C], BF16, f"oh{p}")
            oh_ops.append(nc.vector.tensor_tensor(
                out=t[:], in0=idx_sb[:, (1 + p) * C:(2 + p) * C],
                in1=idx_sb[:, 0:C], op=ALU.is_equal))
            oh.append(t)

        # ---- main matmuls: psum_lin[b, (p,q)] ------------------------
        # DoubleRow fp8: k-tiles are paired (K=256 per matmul, 2 MACs/
        # cell/cycle) halving the matmul+LDWEIGHTS count.  xT is laid
        # out [128f, T, ko, 256b], W chunks [128f, tt, ko, 1024pq].
        # bt-OUTER: bt0's accumulation completes ~5us before bt1's, so
        # its softmax chain and tail overlap bt1's main matmuls; the
        # PE order is pinned with a scheduler hint (Tile otherwise
        # interleaves the two bt's matmul groups).
        # The whole softmax chain lives on ACT: Exp with accum_out
        # replaces the DVE reduce, and lse is subtracted at DRAIN time
        # (DVE drains: fused scalar_tensor_tensor; ACT drains: Identity
        # with per-partition bias) so DVE only builds one-hots and
        # drains.
        DR = mybir.MatmulPerfMode.DoubleRow
        lin = [ps.tile([128, PL, 128], F32, tag="ps", name=f"lin{bt}")
               for bt in (0, 1)]
        linY = [None, None]
        lses = [None, None]
        nlses = [None, None]
        last_mm = [None]
        first_mm = [None]

        main_mms = []

        def emit_main(bt):
            for ch in (0, 1):
                mm = nc.tensor.matmul(
                    lin[bt][:, ch * 4:(ch + 1) * 4, :],
                    ones_ap, biasr[:, ch * 512:(ch + 1) * 512],
                    start=True, stop=False)
                if bt == 0 and first_mm[0] is None:
                    first_mm[0] = mm
                if bt == 1 and last_mm[0] is not None:
                    tile.add_dep_helper(mm.ins, last_mm[0].ins, sync=False,
                                        reason="bt0 main before bt1 main")
            for t in range(KT // 2):
                j, tt = t // (KC // 2), t % (KC // 2)
                for ch in (0, 1):
                    mm = nc.tensor.matmul(
                        lin[bt][:, ch * 4:(ch + 1) * 4, :],
                        xT[:, t, :, bt * 128:(bt + 1) * 128],
                        wkc[j][:, tt, :, ch * 512:(ch + 1) * 512],
                        start=False, stop=(t == KT // 2 - 1),
                        perf_mode=DR)
                    main_mms.append(mm)
            last_mm[0] = mm

        reduce_ops = []
        chain_ops = []

        def emit_chain(bt):
            exps = fresh([128, PL, 128], BF16, f"exps{bt}")
            e = nc.scalar.activation(out=exps[:], in_=lin[bt][:],
                                     func=AF.Exp, scale=1.0 / SCALE)
            sums = fresh([128, PL], F32, f"sums{bt}")
            reduce_ops.append(
                nc.vector.tensor_reduce(out=sums[:], in_=exps[:],
                                        axis=mybir.AxisListType.X,
                                        op=ALU.add))
            ly = fresh([128, PL, 128], BF16, f"linY{bt}")
            lyop = nc.scalar.activation(out=ly[:], in_=lin[bt][:],
                                        func=AF.Copy)
            lse = fresh([128, PL], F32, f"lse{bt}")
            nc.scalar.activation(out=lse[:], in_=sums[:], func=AF.Ln)
            nlse = fresh([128, PL], F32, f"nlse{bt}")
            nc.scalar.mul(nlse[:], lse[:], -1.0)
            linY[bt] = ly
            lses[bt] = lse
            nlses[bt] = nlse
            chain_ops.append((e, lyop))

        emit_main(0)
        emit_chain(0)
        emit_main(1)
        emit_chain(1)

        # bt0's whole ACT chain (Ln/neg/linY) must precede exp1 on the
        # ACT queue, else Ln0 gets stuck behind exp1's data wait
        tile.add_dep_helper(chain_ops[1][0].ins, chain_ops[0][1].ins,
                            sync=False, reason="act order")
        # idx issues scheduled mid-main so the main matmuls' DMA wait
        # targets exclude idx, but idx still lands before the gathers
        tile.add_dep_helper(w_dmas[1].ins, main_mms[0].ins, sync=False,
                            reason="W1 issue after main start")
        for dma in idx_dma:
            tile.add_dep_helper(dma.ins, main_mms[12].ins, sync=False,
                                reason="idx issue mid-main")
        # pin the DVE queue so the softmax reduces are not stuck behind
        # all eight one-hot builds
        tile.add_dep_helper(reduce_ops[0].ins, oh_ops[3].ins, sync=False,
                            reason="dve order")
        tile.add_dep_helper(oh_ops[4].ins, reduce_ops[0].ins, sync=False,
                            reason="dve order")
        tile.add_dep_helper(reduce_ops[1].ins, oh_ops[4].ins, sync=False,
                            reason="dve order")
        tile.add_dep_helper(oh_ops[5].ins, reduce_ops[1].ins, sync=False,
                            reason="dve order")

        # ---- per-bt: transpose -> gather -> drain -> out DMA ---------
        # drain/linT engine coloring by p%4 class (stride-4 PSUM slot
        # rotation makes a gather's WAR partner share its class, so its
        # input dep and WAR dep land on one semaphore).  ACT drains
        # measure ~0.69us vs DVE's ~1.2us, so ACT takes 10 of 16.
        def drain_dve(bt, p):
            return (p % 4) < 2 if bt == 0 else (p % 4) == 0

        for bt in (0, 1):
            tr = ps.tile([128, 2, 4, 128], BF16, tag="ps", name=f"tr{bt}")
            for p in range(PL):
                nc.tensor.transpose(tr[:, p // 4, p % 4, :],
                                    linY[bt][:, p, :], ident[:])
            linT = fresh([128, 2, 4, 128], BF16, f"linT{bt}")
            if bt == 0:
                lt_dve = nc.vector.tensor_copy(out=linT[:, :, 0:2, :],
                                               in_=tr[:, :, 0:2, :])
                lt_act = nc.scalar.activation(out=linT[:, :, 2:4, :],
                                              in_=tr[:, :, 2:4, :],
                                              func=AF.Copy)
                # unblock bt0's gathers: exp1 (chain1) yields the ACT
                # queue to linT0's ACT half; oh5..7 yield DVE likewise
                tile.add_dep_helper(chain_ops[1][0].ins, lt_act.ins,
                                    sync=False, reason="linT0 before exp1")
                tile.add_dep_helper(oh_ops[5].ins, lt_dve.ins,
                                    sync=False, reason="linT0 before oh5")
            else:
                nc.vector.tensor_copy(out=linT[:, :, 0, :],
                                      in_=tr[:, :, 0, :])
                nc.scalar.activation(out=linT[:, :, 1, :],
                                     in_=tr[:, :, 1, :], func=AF.Copy)
                nc.scalar.activation(out=linT[:, :, 2:4, :],
                                     in_=tr[:, :, 2:4, :], func=AF.Copy)

            for p in range(PL):
                po = ps.tile([128, 1024], F32, tag="ps", name=f"po{bt}_{p}")
                lt = linT[:, p // 4, p % 4, :]
                nc.tensor.matmul(po[:, 0:512], lt, oh[p][:, 0:512],
                                 start=True, stop=True)
                nc.tensor.matmul(po[:, 512:1000], lt, oh[p][:, 512:1000],
                                 start=True, stop=True)
                g = fresh([128, C], BF16, f"og{bt}_{p}")
                # drain = gathered 64*logit / 64 - lse, fused per engine
                if drain_dve(bt, p):
                    nc.vector.scalar_tensor_tensor(
                        out=g[:], in0=po[:, 0:1000],
                        scalar=1.0 / SCALE,
                        in1=lses[bt][:, p:p + 1].broadcast_to((128, C)),
                        op0=ALU.mult, op1=ALU.subtract)
                else:
                    nc.scalar.activation(
                        out=g[:], in_=po[:, 0:1000],
                        func=AF.Identity, scale=1.0 / SCALE,
                        bias=nlses[bt][:, p:p + 1])
                bsl = slice(bt * 128, (bt + 1) * 128)
                nc.sync.dma_start(out=out_d[bsl, p:p + 1, :], in_=g[:])

    _install_wait_split(nc)
    return nc


def _install_wait_split(nc):
    """This walrus build encodes at most ONE sync-wait per instruction.
    Legalize at serialization time: any instruction carrying N>1 waits
    gets N-1 wait-only Drain instructions (same engine, so the queue
    stalls identically) inserted in front of it; the instruction keeps
    the last wait.  Semantically identical (serial sem waits)."""
    import json

    orig = nc.to_json_bytes

    def patched():
        m = json.loads(orig())
        for fn in m["functions"]:
            for bb in fn["blocks"]:
                out = []
                for inst in bb["instructions"]:
                    si = inst.get("sync_info")
                    waits = (si or {}).get("on_wait") or []
                    if len(waits) > 1:
                        head, keep = waits[:-1], waits[-1:]
                        for j, w in enumerate(head):
                            out.append({
                                "engine": inst["engine"],
                                "ins": [],
                                "outs": [],
                                "name": f"{inst['name']}-ws{j}",
                                "opcode": "Drain",
                                "sync_info": {
                                    "on_wait": [w],
                                    "on_update": [],
                                },
                            })
                        si["on_wait"] = keep
                    out.append(inst)
                bb["instructions"] = out
        return json.dumps(m).encode()

    nc.to_json_bytes = patched


def _host_inputs(x, W, b, part_idx):
    import ml_dtypes

    f8 = ml_dtypes.float8_e4m3
    bf = ml_dtypes.bfloat16

    # xT[f_sub, t, ko, b] = x[b, (2t+ko)*128 + f_sub]  (DoubleRow pairs)
    xT = np.ascontiguousarray(
        x.reshape(B, KT, 128).transpose(2, 1, 0)       # [128, KT, B]
        .reshape(128, KT // 2, 2, B)).astype(f8)
    ident = np.eye(128, dtype=np.float32).astype(bf)
    iota = np.arange(128, dtype=np.float16)

    in_maps = []
    for i in range(NCORES):
        sl = slice(i * PL, (i + 1) * PL)
        # wm[j, f_sub, tt, ko, p*128+q] = SCALE * W[p, q, k*128+f_sub]
        # with k = j*KC + 2*tt + ko  (DoubleRow pairs)
        wm = np.ascontiguousarray(
            (W[sl] * SCALE).transpose(2, 0, 1)          # [F, PL, Q]
            .reshape(KT // KC, KC, 128, PL * Q)
            .transpose(0, 2, 1, 3)                      # [J, 128, KC, PL*Q]
            .reshape(KT // KC, 128, KC // 2, 2, PL * Q)).astype(f8)
        biasr = np.empty((1, PL * Q + 128), dtype=bf)
        biasr[0, :PL * Q] = (b[sl] * SCALE).reshape(-1).astype(bf)
        biasr[0, PL * Q:] = 1.0
        idxq = np.empty((128, (PL + 1) * C), dtype=np.float16)
        idxq[:, 0:C] = iota[:, None]
        idxq[:, C:] = np.broadcast_to(
            part_idx[sl].astype(np.float16).reshape(1, PL * C),
            (128, PL * C))
        in_maps.append({"xT": xT, "biasr": biasr, "ident": ident,
                        "wm": wm, "idxq": idxq})
    return in_maps


def kernel(x, W, b, part_idx, _trace=False):
    from concourse.bass_utils import run_bass_kernel_spmd

    x = np.asarray(x, dtype=np.float32)
    W = np.asarray(W, dtype=np.float32)
    b = np.asarray(b, dtype=np.float32)
    part_idx = np.asarray(part_idx)

    nc = _build_nc()
    in_maps = _host_inputs(x, W, b, part_idx)
    res = run_bass_kernel_spmd(nc, in_maps, list(range(NCORES)),
                               trace=_trace)
    out = np.concatenate(
        [np.asarray(r["out"], dtype=np.float32) for r in res.results], axis=1)
    if _trace:
        return out, res
    return out
